# Initial kernel scaffold
#
"""DomainBatchNorm Trainium2 kernel.

Math (per sample row r with one-hot domain mask m_r over D=8 domains):
    scale = gammas * rsqrt(pop_vars + eps)            # [D, F]
    shift = betas  - pop_means * scale                # [D, F]
    y[r]  = x[r] * (m_r @ scale) + (m_r @ shift)      # [B, F]

Strategy: pure data-parallel over the batch dim on 8 NeuronCores (4096 rows
per core).  Per 128-row tile, the [128, F] effective scale/shift are produced
on the TensorEngine as mask-tile @ table matmuls (K = D = 8).  The mask is
one-hot so it is exact in bf16; the fp32 scale/shift tables are fed through
the PE as a bf16 hi + bf16 lo split, accumulated in fp32 PSUM, which
reconstructs them to ~2^-18 relative accuracy.  The elementwise
y = x*es + et runs as two fp32 tensor_tensor ops on the VectorEngine.
The kernel is memory-roofline bound: 16 MiB in + 16 MiB out per core.
"""

import numpy as np
import ml_dtypes

B, F, D = 32768, 1024, 8
EPS = 1e-5
N_CORES = 8
ROWS = B // N_CORES          # 4096 rows per core
P = 128                      # partitions / rows per tile
N_TILES = ROWS // P          # 32
HALF = 512                   # fp32 matmul moving-operand max (one PSUM bank)

_NC_CACHE = {}


def _build_nc():
    import concourse.bacc as bacc
    import concourse.tile as tile
    from concourse import mybir

    f32 = mybir.dt.float32
    bf16 = mybir.dt.bfloat16

    nc = bacc.Bacc(
        "TRN2", target_bir_lowering=False, debug=False, num_devices=N_CORES
    )

    x = nc.dram_tensor("x", [ROWS, F], f32, kind="ExternalInput").ap()
    maskT = nc.dram_tensor("maskT", [D, ROWS], bf16, kind="ExternalInput").ap()
    s_hi = nc.dram_tensor("s_hi", [D, F], bf16, kind="ExternalInput").ap()
    s_lo = nc.dram_tensor("s_lo", [D, F], bf16, kind="ExternalInput").ap()
    t_hi = nc.dram_tensor("t_hi", [D, F], bf16, kind="ExternalInput").ap()
    t_lo = nc.dram_tensor("t_lo", [D, F], bf16, kind="ExternalInput").ap()
    y = nc.dram_tensor("y", [ROWS, F], f32, kind="ExternalOutput").ap()

    with tile.TileContext(nc) as tc:
        with (
            tc.tile_pool(name="consts", bufs=1) as consts,
            tc.tile_pool(name="xp", bufs=6) as xp,
            tc.tile_pool(name="tmpp", bufs=3) as tmpp,
            tc.tile_pool(name="outp", bufs=6) as outp,
            tc.tile_pool(name="psp", bufs=2, space="PSUM") as psp,
            tc.tile_pool(name="ptp", bufs=2, space="PSUM") as ptp,
        ):
            mT = consts.tile([D, ROWS], bf16)
            nc.sync.dma_start(out=mT, in_=maskT)
            sh = consts.tile([D, F], bf16)
            nc.sync.dma_start(out=sh, in_=s_hi)
            sl = consts.tile([D, F], bf16)
            nc.sync.dma_start(out=sl, in_=s_lo)
            th = consts.tile([D, F], bf16)
            nc.sync.dma_start(out=th, in_=t_hi)
            tl = consts.tile([D, F], bf16)
            nc.sync.dma_start(out=tl, in_=t_lo)

            for i in range(N_TILES):
                xt = xp.tile([P, F], f32)
                nc.sync.dma_start(out=xt, in_=x[i * P : (i + 1) * P, :])

                w = mT[:, i * P : (i + 1) * P]  # [D, P] one-hot tile (lhsT)
                ps = psp.tile([P, F], f32)      # eff_scale
                pt = ptp.tile([P, F], f32)      # eff_shift
                for j in (0, 1):
                    c = slice(j * HALF, (j + 1) * HALF)
                    nc.tensor.matmul(ps[:, c], lhsT=w, rhs=sh[:, c], start=True, stop=False)
                    nc.tensor.matmul(ps[:, c], lhsT=w, rhs=sl[:, c], start=False, stop=True)
                    nc.tensor.matmul(pt[:, c], lhsT=w, rhs=th[:, c], start=True, stop=False)
                    nc.tensor.matmul(pt[:, c], lhsT=w, rhs=tl[:, c], start=False, stop=True)

                tmp = tmpp.tile([P, F], f32)
                nc.vector.tensor_mul(tmp, xt, ps)
                ot = outp.tile([P, F], f32)
                nc.vector.tensor_add(ot, tmp, pt)
                nc.sync.dma_start(out=y[i * P : (i + 1) * P, :], in_=ot)

    nc.compile()
    return nc


def _get_nc():
    if "nc" not in _NC_CACHE:
        _NC_CACHE["nc"] = _build_nc()
    return _NC_CACHE["nc"]


def _prep_in_maps(inputs, mask, gammas, betas, pop_means, pop_vars):
    bf = ml_dtypes.bfloat16
    # Fold the per-domain params into scale/shift tables (tiny [D, F] work),
    # in float64 so the bf16 hi/lo split captures the true value.
    scale64 = gammas.astype(np.float64) / np.sqrt(pop_vars.astype(np.float64) + EPS)
    shift64 = betas.astype(np.float64) - pop_means.astype(np.float64) * scale64
    s_hi = scale64.astype(bf)
    s_lo = (scale64 - s_hi.astype(np.float64)).astype(bf)
    t_hi = shift64.astype(bf)
    t_lo = (shift64 - t_hi.astype(np.float64)).astype(bf)

    maskT = np.ascontiguousarray(mask.astype(bf).T)  # one-hot: exact in bf16

    in_maps = []
    for c in range(N_CORES):
        r0, r1 = c * ROWS, (c + 1) * ROWS
        in_maps.append(
            {
                "x": np.ascontiguousarray(inputs[r0:r1]),
                "maskT": np.ascontiguousarray(maskT[:, r0:r1]),
                "s_hi": s_hi,
                "s_lo": s_lo,
                "t_hi": t_hi,
                "t_lo": t_lo,
            }
        )
    return in_maps


def kernel(inputs, mask, gammas, betas, pop_means, pop_vars, _trace=False, **_tr_kw):
    from concourse.bass_utils import run_bass_kernel_spmd

    in_maps = _prep_in_maps(inputs, mask, gammas, betas, pop_means, pop_vars)
    nc = _get_nc()
    res = run_bass_kernel_spmd(
        nc, in_maps, list(range(N_CORES)), trace=_trace, **_tr_kw
    )
    out = np.concatenate([res.results[c]["y"] for c in range(N_CORES)], axis=0)
    if _trace:
        kernel.last_results = res
    return out


# revision 5
# speedup vs baseline: 1.3382x; 1.3382x over previous
"""DomainBatchNorm Trainium2 kernel.

Math (per sample row r with one-hot domain mask m_r over D=8 domains):
    scale = gammas * rsqrt(pop_vars + eps)            # [D, F]
    shift = betas  - pop_means * scale                # [D, F]
    y[r]  = x[r] * (m_r @ scale) + (m_r @ shift)      # [B, F]

Strategy: pure data-parallel over the batch dim on 8 NeuronCores (4096 rows
per core).  Per 128-row tile, the [128, F] effective scale/shift are produced
on the TensorEngine as mask-tile @ table matmuls (K = D = 8).  The mask is
one-hot so it is exact in bf16; the fp32 scale/shift tables are fed through
the PE as a bf16 hi + bf16 lo split, accumulated in fp32 PSUM, which
reconstructs them to ~2^-18 relative accuracy.  The elementwise
y = x*es + et runs as two fp32 tensor_tensor ops on the VectorEngine.
The kernel is memory-roofline bound: 16 MiB in + 16 MiB out per core.
"""

import numpy as np
import ml_dtypes

B, F, D = 32768, 1024, 8
EPS = 1e-5
N_CORES = 8
ROWS = B // N_CORES          # 4096 rows per core
P = 128                      # partitions / rows per tile
N_TILES = ROWS // P          # 32
HALF = 512                   # fp32 matmul moving-operand max (one PSUM bank)

_NC_CACHE = {}


def _build_nc(reps=1):
    import concourse.bacc as bacc
    import concourse.tile as tile
    from concourse import mybir

    f32 = mybir.dt.float32
    bf16 = mybir.dt.bfloat16

    nc = bacc.Bacc(
        "TRN2", target_bir_lowering=False, debug=False, num_devices=N_CORES
    )

    x = nc.dram_tensor("x", [ROWS, F], f32, kind="ExternalInput").ap()
    maskT = nc.dram_tensor("maskT", [D, ROWS], bf16, kind="ExternalInput").ap()
    s_hi = nc.dram_tensor("s_hi", [D, F], bf16, kind="ExternalInput").ap()
    s_lo = nc.dram_tensor("s_lo", [D, F], bf16, kind="ExternalInput").ap()
    t_hi = nc.dram_tensor("t_hi", [D, F], bf16, kind="ExternalInput").ap()
    t_lo = nc.dram_tensor("t_lo", [D, F], bf16, kind="ExternalInput").ap()
    y = nc.dram_tensor("y", [ROWS, F], f32, kind="ExternalOutput").ap()

    with tile.TileContext(nc) as tc:
        with (
            tc.tile_pool(name="consts", bufs=1) as consts,
            tc.tile_pool(name="xp", bufs=6) as xp,
            tc.tile_pool(name="tmpp", bufs=3) as tmpp,
            tc.tile_pool(name="outp", bufs=6) as outp,
            tc.tile_pool(name="psp", bufs=2, space="PSUM") as psp,
            tc.tile_pool(name="ptp", bufs=2, space="PSUM") as ptp,
        ):
            mT = consts.tile([D, ROWS], bf16)
            nc.sync.dma_start(out=mT, in_=maskT)
            sh = consts.tile([D, F], bf16)
            nc.sync.dma_start(out=sh, in_=s_hi)
            sl = consts.tile([D, F], bf16)
            nc.sync.dma_start(out=sl, in_=s_lo)
            th = consts.tile([D, F], bf16)
            nc.sync.dma_start(out=th, in_=t_hi)
            tl = consts.tile([D, F], bf16)
            nc.sync.dma_start(out=tl, in_=t_lo)

            def body():
                for i in range(N_TILES):
                    xt = xp.tile([P, F], f32)
                    nc.sync.dma_start(out=xt, in_=x[i * P : (i + 1) * P, :])

                    w = mT[:, i * P : (i + 1) * P]  # [D, P] one-hot tile (lhsT)
                    ps = psp.tile([P, F], f32)      # eff_scale
                    pt = ptp.tile([P, F], f32)      # eff_shift
                    for j in (0, 1):
                        c = slice(j * HALF, (j + 1) * HALF)
                        nc.tensor.matmul(ps[:, c], lhsT=w, rhs=sh[:, c], start=True, stop=False)
                        nc.tensor.matmul(ps[:, c], lhsT=w, rhs=sl[:, c], start=False, stop=True)
                        nc.tensor.matmul(pt[:, c], lhsT=w, rhs=th[:, c], start=True, stop=False)
                        nc.tensor.matmul(pt[:, c], lhsT=w, rhs=tl[:, c], start=False, stop=True)

                    tmp = tmpp.tile([P, F], f32)
                    nc.vector.tensor_mul(tmp, xt, ps)
                    ot = outp.tile([P, F], f32)
                    nc.vector.tensor_add(ot, tmp, pt)
                    nc.sync.dma_start(out=y[i * P : (i + 1) * P, :], in_=ot)

            if reps == 1:
                body()
            else:
                # bench mode: repeat the whole pipeline in a HW loop so one
                # NEFF execution carries `reps` kernel-equivalents of work
                with tc.For_i(0, reps, 1):
                    body()

    nc.compile()
    return nc


def _get_nc(reps=1):
    if reps not in _NC_CACHE:
        _NC_CACHE[reps] = _build_nc(reps)
    return _NC_CACHE[reps]


def _prep_in_maps(inputs, mask, gammas, betas, pop_means, pop_vars):
    bf = ml_dtypes.bfloat16
    # Fold the per-domain params into scale/shift tables (tiny [D, F] work),
    # in float64 so the bf16 hi/lo split captures the true value.
    scale64 = gammas.astype(np.float64) / np.sqrt(pop_vars.astype(np.float64) + EPS)
    shift64 = betas.astype(np.float64) - pop_means.astype(np.float64) * scale64
    s_hi = scale64.astype(bf)
    s_lo = (scale64 - s_hi.astype(np.float64)).astype(bf)
    t_hi = shift64.astype(bf)
    t_lo = (shift64 - t_hi.astype(np.float64)).astype(bf)

    maskT = np.ascontiguousarray(mask.astype(bf).T)  # one-hot: exact in bf16

    in_maps = []
    for c in range(N_CORES):
        r0, r1 = c * ROWS, (c + 1) * ROWS
        in_maps.append(
            {
                "x": np.ascontiguousarray(inputs[r0:r1]),
                "maskT": np.ascontiguousarray(maskT[:, r0:r1]),
                "s_hi": s_hi,
                "s_lo": s_lo,
                "t_hi": t_hi,
                "t_lo": t_lo,
            }
        )
    return in_maps


def kernel(inputs, mask, gammas, betas, pop_means, pop_vars, _trace=False, **_tr_kw):
    from concourse.bass_utils import run_bass_kernel_spmd

    in_maps = _prep_in_maps(inputs, mask, gammas, betas, pop_means, pop_vars)
    nc = _get_nc()
    res = run_bass_kernel_spmd(
        nc, in_maps, list(range(N_CORES)), trace=_trace, **_tr_kw
    )
    out = np.concatenate([res.results[c]["y"] for c in range(N_CORES)], axis=0)
    if _trace:
        kernel.last_results = res
    return out
